# revision 1
# baseline (speedup 1.0000x reference)
"""Trainium2 Bass kernel for CapsuleLayer dynamic routing.

Math (faithful to the reference, including the torch size-1-dim quirk):
  u[b,c,n,o] = sum_i W[c,n,o,i] * x[b,n,i]
  iter0: c0 = 1/C                -> s0[b,c,o] = sum_n u / C
  iter1: e = exp(u * v0)         -> s1 = sum_n u*e / sum_c e
  iter2: e = exp(u * (v0+v1))    -> s2 = sum_n u*e / sum_c e
  out   = squash(s2),  squash(s) = s^3 / ((1+s^2) * sqrt(s^2+eps))

Distribution: shard N (=1152) over 8 cores (144 routes each). Each
iteration needs only an all-reduce of s (64*32*64 floats) across cores.

Per-core layout:
  - phase A streams W once, computes u via PE (lhsT = W-block [i, 2c*o],
    moving = xT_n [i, b]), accumulates u tiles in PSUM, evicts to a
    staging tile [128=(c2,o), (b, n)] per c-pair, spills to DRAM.
    s0 partials come from a free-dim tensor_reduce of the staging tile.
  - iters 1/2 stream u back per (chunk of CB batches, cpair):
    t = clamp(u*V, <=85) (tensor_scalar per b), e = exp(t) (ACT),
    D = sum_c e (PE matmul with 0/1 lhsT, PSUM-accumulated over cpairs,
    replicated on both partition halves), r = 1/D, q = e*r, p = q*u,
    s-slice = reduce_n p.
"""

import os
import sys

import numpy as np

sys.path.insert(0, "/opt/trn_rl_repo")

import concourse.bacc as bacc
import concourse.mybir as mybir
import concourse.tile as tile
from concourse.bass_utils import run_bass_kernel_spmd

F32 = mybir.dt.float32
BF16 = mybir.dt.bfloat16
F16 = mybir.dt.float16
AX = mybir.AxisListType
ALU = mybir.AluOpType
ACTF = mybir.ActivationFunctionType

EPS = 1e-8
CLAMP = 85.0


class Cfg:
    def __init__(self, B=64, C=32, O=64, I=64, N=1152, n_cores=8, CB=4,
                 it_bf16=False, u_bf=None, e_bf=None, r_bf=None, q_bf=None):
        self.B, self.C, self.O, self.I, self.N = B, C, O, I, N
        d = lambda v: (it_bf16 if v is None else v)
        # u and the exp argument te need fp16 (10-bit mantissa) so that
        # delta-t on exp args (|t| up to ~129) stays ~0.06; e spans e^(+-85)
        # so it must be bf16 (fp16 would overflow). 2-byte dtypes keep the
        # DVE 2x/4x modes.
        self.u_dt = F16 if d(u_bf) else F32
        self.te_dt = F16 if d(e_bf) else F32
        self.e_dt = BF16 if d(e_bf) else F32
        self.r_dt = BF16 if d(r_bf) else F32
        self.q_dt = BF16 if d(q_bf) else F32
        self.it_dt = BF16 if it_bf16 else F32
        self.mm_max = 1024 if d(e_bf) else 512
        self.n_cores = n_cores
        self.N_LOC = N // n_cores
        self.CP = C // 2
        self.P = 128
        self.CB = CB                      # batches per routing chunk
        assert B % CB == 0
        self.NCHUNK = B // CB
        self.COLS = CB * self.N_LOC       # free columns per routing tile
        ne = max(1, 512 // B)             # n's per phase-A PSUM eviction
        while self.N_LOC % ne:
            ne -= 1
        self.NE = ne
        self.EB = self.N_LOC // ne


def build_module(cfg: Cfg):
    B, C, O, I = cfg.B, cfg.C, cfg.O, cfg.I
    N_LOC, CP, P = cfg.N_LOC, cfg.CP, cfg.P
    CB, NCHUNK, COLS = cfg.CB, cfg.NCHUNK, cfg.COLS
    NE, EB = cfg.NE, cfg.EB

    nc = bacc.Bacc(
        "TRN2", target_bir_lowering=False, debug=False, num_devices=cfg.n_cores
    )
    w3 = nc.dram_tensor("w3", [CP, N_LOC, I, 2 * O], F32, kind="ExternalInput").ap()
    xt = nc.dram_tensor("xt", [I, N_LOC, B], F32, kind="ExternalInput").ap()
    ones2_in = nc.dram_tensor("ones2", [P, P], cfg.e_dt, kind="ExternalInput").ap()
    vout = nc.dram_tensor("vout", [P, CP * B], F32, kind="ExternalOutput").ap()

    groups = [list(range(cfg.n_cores))]

    with tile.TileContext(nc) as tc:
        with (
            tc.tile_pool(name="glob", bufs=1) as glob,
            tc.tile_pool(name="dram", bufs=1, space="DRAM") as dram,
        ):
            u_dram = dram.tile([CP, P, B * N_LOC], cfg.u_dt)
            ones_sb = glob.tile([P, P], cfg.e_dt)
            nc.sync.dma_start(ones_sb[:, :], ones2_in)
            s_acc = glob.tile([P, CP * B], F32)
            uab = glob.tile([P, CP * B], F32)
            eps_sb = glob.tile([P, 1], F32)
            nc.vector.memset(eps_sb, EPS)

            # ---------------- phase A: einsum + s0 ----------------
            with (
                tc.tile_pool(name="xp", bufs=1) as xp,
                tc.tile_pool(name="wp", bufs=4) as wp,
                tc.tile_pool(name="stgp", bufs=2) as stgp,
                tc.tile_pool(name="paps", bufs=2, space="PSUM") as paps,
            ):
                x_sb = xp.tile([I, N_LOC * B], F32)
                nc.sync.dma_start(
                    x_sb.rearrange("i (n b) -> i n b", n=N_LOC), xt
                )
                x3 = x_sb.rearrange("i (n b) -> i n b", n=N_LOC)
                for cp in range(CP):
                    stg = stgp.tile([P, B * N_LOC], cfg.u_dt, tag="stg")
                    stg3 = stg.rearrange("p (b n) -> p b n", b=B)
                    for eb in range(EB):
                        ups = paps.tile([P, NE * B], F32, tag="ups")
                        wt = wp.tile([I, NE * 2 * O], F32, tag="wt")
                        nc.sync.dma_start(
                            wt.rearrange("i (j m) -> i j m", j=NE),
                            w3[cp, eb * NE : (eb + 1) * NE].rearrange(
                                "j i m -> i j m"
                            ),
                        )
                        for j in range(NE):
                            nc.tensor.matmul(
                                ups[:, j * B : (j + 1) * B],
                                wt[:, j * 2 * O : (j + 1) * 2 * O],
                                x3[:, eb * NE + j, :],
                                start=True,
                                stop=True,
                            )
                        nc.scalar.copy(
                            stg3[:, :, eb * NE : (eb + 1) * NE],
                            ups.rearrange("p (j b) -> p b j", j=NE),
                        )
                    nc.vector.tensor_reduce(
                        s_acc[:, cp * B : (cp + 1) * B],
                        stg3,
                        axis=AX.X,
                        op=ALU.add,
                    )
                    nc.vector.tensor_reduce(
                        uab[:, cp * B : (cp + 1) * B],
                        stg3,
                        axis=AX.X,
                        op=ALU.max,
                        apply_absolute_value=True,
                    )
                    nc.sync.dma_start(u_dram[cp], stg)

            # ---------------- helpers ----------------
            def all_reduce(s_sb, out_sb, tag):
                bin_ = dram.tile([P, CP * B], F32, name=f"arin{tag}")
                bout = dram.tile([P, CP * B], F32, name=f"arout{tag}")
                nc.gpsimd.dma_start(bin_[:, :], s_sb[:, :])
                nc.gpsimd.collective_compute(
                    "AllReduce",
                    ALU.add,
                    replica_groups=groups,
                    ins=[bin_.opt()],
                    outs=[bout.opt()],
                )
                nc.gpsimd.dma_start(out_sb[:, :], bout[:, :])

            def squash(v_out, s_in, scale, pool, tag):
                shp = [P, CP * B]
                sc = pool.tile(shp, F32, name=f"sc{tag}", tag="sq_sc", bufs=1)
                sq = pool.tile(shp, F32, name=f"sq{tag}", tag="sq_sq", bufs=1)
                rt = pool.tile(shp, F32, name=f"rt{tag}", tag="sq_rt", bufs=1)
                den = pool.tile(shp, F32, name=f"den{tag}", tag="sq_den", bufs=1)
                rec = pool.tile(shp, F32, name=f"rec{tag}", tag="sq_rec", bufs=1)
                num = pool.tile(shp, F32, name=f"num{tag}", tag="sq_num", bufs=1)
                if scale != 1.0:
                    nc.vector.tensor_scalar_mul(sc, s_in, scale)
                else:
                    sc = s_in
                nc.scalar.square(sq, sc)
                nc.scalar.activation(rt, sq, ACTF.Sqrt, bias=eps_sb)
                nc.vector.scalar_tensor_tensor(
                    den, sq, 1.0, rt, op0=ALU.add, op1=ALU.mult
                )
                nc.vector.reciprocal(rec, den)
                nc.vector.tensor_mul(num, sc, sq)
                nc.vector.tensor_mul(v_out, num, rec)

            sglob = glob.tile([P, CP * B], F32, tag="sg", bufs=2)
            v0 = glob.tile([P, CP * B], F32, tag="vv", bufs=4)

            all_reduce(s_acc, sglob, "s0")
            squash(v0, sglob, 1.0 / C, glob, "v0")
            vv = v0

            # ---------------- routing iterations ----------------
            def splitmm(total):
                out, c0 = [], 0
                while c0 < total:
                    out.append((c0, min(c0 + cfg.mm_max, total)))
                    c0 = out[-1][1]
                return out

            def build_bias(vvt, tag):
                # bias2[p, b] = -relu(max_c(|V[b,c,o]| * Uabs[b,c,o]) - CLAMP)
                # (c-independent => softmax-invariant shift of exp args)
                av = glob.tile([P, CP * B], F32, name=f"av{tag}", tag="mb_av", bufs=1)
                pr = glob.tile([P, CP * B], F32, name=f"pr{tag}", tag="mb_pr", bufs=1)
                mx1 = glob.tile([P, B], F32, name=f"mx1{tag}", tag="mb_mx1", bufs=1)
                tmph = glob.tile([O, B], F32, name=f"tmph{tag}", tag="mb_tmph", bufs=1)
                bias2 = glob.tile([P, B], F32, name=f"b2{tag}", tag="mb_b2", bufs=1)
                nc.scalar.activation(av, vvt, ACTF.Abs)
                nc.vector.tensor_mul(pr, av, uab)
                nc.vector.tensor_reduce(
                    mx1,
                    pr.rearrange("p (cp b) -> p b cp", b=B),
                    axis=AX.X,
                    op=ALU.max,
                )
                nc.sync.dma_start(tmph[:, :], mx1[O : 2 * O, :])
                nc.vector.tensor_max(mx1[0:O, :], mx1[0:O, :], tmph[:, :])
                nc.vector.tensor_scalar(
                    mx1[0:O, :], mx1[0:O, :], CLAMP, 0.0,
                    op0=ALU.subtract, op1=ALU.max,
                )
                nc.vector.tensor_scalar_mul(bias2[0:O, :], mx1[0:O, :], -1.0)
                nc.sync.dma_start(bias2[O : 2 * O, :], bias2[0:O, :])
                return bias2

            for it in (1, 2):
                bias2 = build_bias(vv, f"it{it}")
                with (
                    tc.tile_pool(name=f"up{it}", bufs=CP + 2) as up,
                    tc.tile_pool(name=f"ep{it}", bufs=CP + 2) as ep,
                    tc.tile_pool(name=f"qp{it}", bufs=3) as qp,
                    tc.tile_pool(name=f"rp{it}", bufs=2) as rp,
                    tc.tile_pool(name=f"dps{it}", bufs=2, space="PSUM") as dpsp,
                ):
                    for ch in range(NCHUNK):
                        dps = dpsp.tile([P, COLS], F32, tag="dps")
                        uts, ets = [], []
                        for cp in range(CP):
                            u_t = up.tile([P, COLS], cfg.u_dt, tag="u")
                            nc.sync.dma_start(
                                u_t[:, :],
                                u_dram[cp][:, ch * COLS : (ch + 1) * COLS],
                            )
                            e_t = ep.tile([P, COLS], cfg.e_dt, tag="e")
                            for bb in range(CB):
                                b = ch * CB + bb
                                nc.scalar.activation(
                                    e_t[:, bb * N_LOC : (bb + 1) * N_LOC],
                                    u_t[:, bb * N_LOC : (bb + 1) * N_LOC],
                                    ACTF.Exp,
                                    bias=bias2[:, b : b + 1],
                                    scale=vv[:, cp * B + b : cp * B + b + 1],
                                )
                            for c0, c1 in splitmm(COLS):
                                nc.tensor.matmul(
                                    dps[:, c0:c1],
                                    ones_sb,
                                    e_t[:, c0:c1],
                                    start=(cp == 0),
                                    stop=(cp == CP - 1),
                                )
                            uts.append(u_t)
                            ets.append(e_t)
                        r2 = rp.tile([P, COLS], cfg.r_dt, tag="r2")
                        with nc.allow_low_precision(
                            reason="softmax denom reciprocal stored bf16"
                        ):
                            nc.vector.reciprocal(r2[:, :], dps[:, :])
                        for cp in range(CP):
                            q_t = qp.tile([P, COLS], cfg.q_dt, tag="q")
                            nc.vector.tensor_mul(q_t, ets[cp], r2)
                            p_t = qp.tile([P, COLS], cfg.q_dt, tag="p")
                            # fused p = q*u and s = sum_n p via accum_out,
                            # one op per batch column block
                            for bb in range(CB):
                                sl = slice(bb * N_LOC, (bb + 1) * N_LOC)
                                nc.vector.scalar_tensor_tensor(
                                    p_t[:, sl],
                                    q_t[:, sl],
                                    1.0,
                                    uts[cp][:, sl],
                                    op0=ALU.mult,
                                    op1=ALU.mult,
                                    accum_out=s_acc[
                                        :,
                                        cp * B + ch * CB + bb
                                        : cp * B + ch * CB + bb + 1,
                                    ],
                                )
                if it == 1:
                    s1g = glob.tile([P, CP * B], F32, tag="sg", bufs=2)
                    v1 = glob.tile([P, CP * B], F32, tag="vv", bufs=4)
                    all_reduce(s_acc, s1g, "s1")
                    squash(v1, s1g, 1.0, glob, "v1")
                    vv2 = glob.tile([P, CP * B], F32, tag="vv", bufs=4)
                    nc.vector.tensor_add(vv2, v0, v1)
                    vv = vv2
                else:
                    s2g = glob.tile([P, CP * B], F32, tag="sg", bufs=2)
                    v2 = glob.tile([P, CP * B], F32, tag="vv", bufs=4)
                    all_reduce(s_acc, s2g, "s2")
                    squash(v2, s2g, 1.0, glob, "v2")
                    nc.sync.dma_start(vout, v2[:, :])

    nc.compile()
    return nc


# ---------------------------------------------------------------------------
# host side
# ---------------------------------------------------------------------------

def prep_inputs(x: np.ndarray, route_weights: np.ndarray, cfg: Cfg):
    """Full inputs -> per-core in_maps (host-side re-layout is part of the
    sharding step; it is not in the measured device time)."""
    B, C, O, I, N = cfg.B, cfg.C, cfg.O, cfg.I, cfg.N
    CP, N_LOC, P = cfg.CP, cfg.N_LOC, cfg.P

    rw5 = route_weights.reshape(CP, 2, N, O, I)
    w3_full = np.ascontiguousarray(
        rw5.transpose(0, 2, 4, 1, 3).reshape(CP, N, I, 2 * O)
    )
    xt_full = np.ascontiguousarray(x.transpose(2, 1, 0))  # [I, N, B]

    k = np.arange(P)
    ones2 = (k[:, None] % O == k[None, :] % O).astype(
        np.float32 if cfg.e_dt == F32 else __import__("ml_dtypes").bfloat16
    )

    in_maps = []
    for core in range(cfg.n_cores):
        sl = slice(core * N_LOC, (core + 1) * N_LOC)
        in_maps.append(
            {
                "w3": np.ascontiguousarray(w3_full[:, sl]),
                "xt": np.ascontiguousarray(xt_full[:, sl]),
                "ones2": ones2,
            }
        )
    return in_maps


def postprocess(vout: np.ndarray, cfg: Cfg) -> np.ndarray:
    # vout [P=(c2,o), CP*B] -> v[b, 2cp+c2, o]
    B, C, O, CP = cfg.B, cfg.C, cfg.O, cfg.CP
    v = vout.reshape(2, O, CP, B).transpose(3, 2, 0, 1).reshape(B, C, O)
    return np.ascontiguousarray(v)


_CACHE = {}


def _get_module(cfg: Cfg):
    key = (cfg.B, cfg.C, cfg.O, cfg.I, cfg.N, cfg.n_cores, cfg.CB,
           str(cfg.u_dt), str(cfg.e_dt), str(cfg.r_dt), str(cfg.q_dt))
    if key not in _CACHE:
        _CACHE[key] = build_module(cfg)
    return _CACHE[key]


def kernel(x: np.ndarray, route_weights: np.ndarray) -> np.ndarray:
    cfg = Cfg()
    nc = _get_module(cfg)
    in_maps = prep_inputs(np.asarray(x), np.asarray(route_weights), cfg)
    res = run_bass_kernel_spmd(
        nc, in_maps, core_ids=list(range(cfg.n_cores)),
        trace=bool(int(os.environ.get("KERNEL_TRACE", "0"))),
    )
    out = postprocess(res.results[0]["vout"], cfg)
    if res.exec_time_ns is not None:
        kernel.last_exec_time_ns = res.exec_time_ns
    return out


kernel.last_exec_time_ns = None

